# revision 18
# baseline (speedup 1.0000x reference)
"""DeepWalk loss kernel for 8 Trainium2 NeuronCores.

Strategy: data-parallel over the 512 walks (64 walks per core). Each core
compacts the referenced embedding rows into a DRAM token table in FP8
(node||ctx, 256B per token, values pre-scaled by S=128) via windowed
dma_gather (windows of 32768 rows so indices fit int16; negative-padded
single-packet calls so pad slots emit no descriptors). Pair operands are
fetched with 1024-index single-packet HBM dma_gather calls in row layout
(pairs on partitions) at 128B/256B per descriptor — half the bytes of the
bf16 variant, and the gathers are bandwidth-bound. Gathered fp8 tiles are
upcast to bf16 on the scalar engine, multiplied on DVE (2x mode) and
reduced along the embedding axis into per-stream score strips; softplus is
evaluated with the Exp and Ln LUTs on the scalar engine with the 1/S^2
score rescale folded into the Exp scale and a fused row-sum accumulator.
Host sums the 8x[128] partial sums and divides by the pair count.
"""

import os
import sys

import numpy as np
import ml_dtypes

sys.path.insert(0, "/opt/trn_rl_repo")

import concourse.bacc as bacc
import concourse.bass as bass
import concourse.mybir as mybir
import concourse.tile as tile
from concourse import library_config
from concourse.bass_utils import run_bass_kernel_spmd
from concourse._compat import with_exitstack
from concourse.tile import add_dep_helper

BF16 = ml_dtypes.bfloat16
E8 = ml_dtypes.float8_e4m3

# Problem constants (hardcoded per the harness contract).
EMB_DIM = 128
WALK_LEN = 40
WINDOW = 5
NEG_SIZE = 5
NUM_NODES = 1_000_000
BATCH = 512
N_CORES = 8

NB_CORE = BATCH // N_CORES            # 64 walks per core
NLOC = NB_CORE * WALK_LEN             # 2560 local walk positions
GTOK = BATCH * WALK_LEN               # 20480 global walk positions
P = 128
WIN_ROWS = 32768                      # int16-addressable window of the table
WCALL = 1024                          # idx per gather call (single-packet max)
FSCALE = 128.0                        # fp8 pre-scale of embedding values
PADV = 448.0                          # fp8 max-magnitude pad value
SCALE2 = 1.0 / (FSCALE * FSCALE)      # score rescale folded into Act

def _pair_indices():
    src, dst = [], []
    for i in range(WALK_LEN):
        for j in range(max(0, i - WINDOW), i):
            src.append(j); dst.append(i)
        for j in range(i + 1, min(WALK_LEN, i + 1 + WINDOW)):
            src.append(j); dst.append(i)
    return np.asarray(src, dtype=np.int64), np.asarray(dst, dtype=np.int64)

_SRC, _DST = _pair_indices()
NUM_PAIRS = _SRC.shape[0]             # 370
POS_CORE = NB_CORE * NUM_PAIRS        # 23680 positive pairs per core
NPAD = 24064                          # pairs per stream, padded to 128 (188 cols)
NCOLS = NPAD // P                     # 188 score columns per stream
CALLS = [WCALL] * (NPAD // WCALL) + ([NPAD % WCALL] if NPAD % WCALL else [])
N_STREAMS = 6                         # pos, neg j=0..4
IDX_COLS = NPAD // 16                 # 1504 idx columns per list
N_LISTS = 7                           # AB, C, D0..D4

# --- experiment knobs (timing/ablation; defaults = production) ---
NO_COMPUTE = False      # skip mult/reduce/softplus (gathers only)
NO_MAIN = False         # skip the main pair gathers (build only)
QUEUES = 4              # SWDGE queues to round-robin gathers over
FAKE_WINDOWS = 0        # timing mode: N fake windows over a small vocab
GRAM = True             # pos scores via per-walk TensorE Gram (no C gathers)
BIGB = 900.0 / SCALE2   # mask bias: softplus(-(s+BIGB)*SCALE2) ~= 0
OWN_CALLS = [1024, 1024, 512]  # own-walk token gather calls (2560 tokens)
AB_BUFS = 10            # AB-tile pool depth (gather pipeline)
D_BUFS = 20             # D-tile pool depth
UP_BUFS = 12            # upcast-tile pool depth


def _wrap16(a):
    """int16 list [N] -> [128, N/16] dma_gather idx layout (16-wrap, 8x replicated)."""
    a = a.astype(np.int16)
    t = a.reshape(-1, 16).T          # [16, N/16]
    return np.tile(t, (8, 1)).copy() # [128, N/16]


def _plan(fw, vocab):
    """Window build plan: group the 20480 walk rows by table window.

    Each window is gathered with one WCALL(=1024)-index single-packet call
    straight into DRAM-table staging; trailing indices are -1 (descriptor
    skipped). Token ids are padw*i + slot.

    Returns (padw, bases, widx_lists, counts, pos).
    """
    if FAKE_WINDOWS:
        nwin = FAKE_WINDOWS
        win = np.argsort(np.argsort(fw, kind="stable"), kind="stable") % nwin
        bases = [0] * nwin
    else:
        nwin = (vocab + WIN_ROWS - 1) // WIN_ROWS
        win = fw // WIN_ROWS
        bases = [WIN_ROWS * i for i in range(nwin)]
    counts = np.bincount(win, minlength=nwin)
    padw = 896
    assert counts.max() <= min(padw, WCALL), f"window overflow: {counts.max()}"
    pos = np.empty(GTOK, dtype=np.int32)
    widx_lists = []
    for i in range(nwin):
        ws = np.nonzero(win == i)[0]
        ws = ws[np.argsort(fw[ws], kind="stable")]  # ascending rows: HBM locality
        pos[ws] = padw * i + np.arange(len(ws), dtype=np.int32)
        lst = np.full(WCALL, -1, dtype=np.int32)
        lst[:len(ws)] = fw[ws] - bases[i]
        widx_lists.append(lst)
    return padw, bases, widx_lists, tuple(int(c) for c in counts), pos


def _host_prepare(batch_walk, neg_idx_dst, node_embed, context_embed):
    """Sharding/index prep. Index arithmetic + dtype casts only."""
    fw = np.asarray(batch_walk).reshape(-1).astype(np.int32)       # [20480]
    neg = np.asarray(neg_idx_dst).astype(np.int32)                 # [947200]
    vocab = int(np.asarray(node_embed).shape[0])

    tok = np.empty((vocab, 2 * EMB_DIM), dtype=E8)
    tok[:, :EMB_DIM] = (np.asarray(node_embed) * FSCALE).astype(E8)
    tok[:, EMB_DIM:] = (np.asarray(context_embed) * FSCALE).astype(E8)

    padw, bases, widx_lists, counts, pos = _plan(fw, vocab)
    nwin = len(bases)
    ntokb = padw * nwin
    pad_x, pad_y, pad_z = ntokb, ntokb + 1, ntokb + 2
    assert ntokb + 3 <= 32768, "token ids must fit int16"

    # pads: AB->X, C->Y, D->Z.  pos pad score = node_Y*ctx_X = +PADV^2;
    # neg pad score = node_X*ctx_Z = -PADV^2. softplus(-PADV^2/S^2) ~= 0.
    ptok = np.zeros((3, 2 * EMB_DIM), dtype=E8)
    ptok[0, 0] = PADV; ptok[0, EMB_DIM] = PADV   # X: node=448, ctx=448
    ptok[1, 0] = PADV                            # Y: node=448
    ptok[2, EMB_DIM] = -PADV                     # Z: ctx=-448

    widx = np.concatenate([_wrap16(a) for a in widx_lists], axis=1)

    bl = np.repeat(np.arange(NB_CORE, dtype=np.int32), NUM_PAIRS)
    qq = np.tile(np.arange(NUM_PAIRS, dtype=np.int32), NB_CORE)
    npad = NPAD - POS_CORE

    # constants for the Gram pos path
    ident = np.eye(P, dtype=BF16)
    ii, jj = np.meshgrid(np.arange(WALK_LEN), np.arange(WALK_LEN), indexing="ij")
    m40 = (np.abs(ii - jj) <= WINDOW) & (ii != jj)       # valid (dst,src) pairs
    bmask = np.where(np.tile(~m40, (1, 12)), BIGB, 0.0).astype(np.float32)

    in_maps = []
    for k in range(N_CORES):
        wloc = k * NLOC  # this core's batches start at walk position k*2560
        ab_t = pos[wloc + bl * WALK_LEN + _DST[qq].astype(np.int32)]
        c_t = pos[wloc + bl * WALK_LEN + _SRC[qq].astype(np.int32)]
        ab = np.concatenate([ab_t, np.full(npad, pad_x, np.int32)])
        cc = np.concatenate([c_t, np.full(npad, pad_y, np.int32)])
        negk = neg[k * POS_CORE * NEG_SIZE:(k + 1) * POS_CORE * NEG_SIZE]
        negk = negk.reshape(POS_CORE, NEG_SIZE)
        lists = [ab, cc]
        for j in range(NEG_SIZE):
            dj = np.concatenate([pos[negk[:, j]], np.full(npad, pad_z, np.int32)])
            lists.append(dj)
        gidx = np.concatenate([_wrap16(a) for a in lists], axis=1)  # [128, 7*1504]
        own = np.full(sum(OWN_CALLS), -1, np.int32)
        own[:NLOC] = pos[wloc:wloc + NLOC]
        oidx = _wrap16(own)
        in_maps.append({"tok": tok, "widx": widx, "gidx": gidx, "ptok": ptok,
                        "oidx": oidx, "ident": ident, "bmask": bmask})
    return in_maps, padw, tuple(bases), counts


@with_exitstack
def _body(ctx, tc, nc, tok_t, widx_t, gidx_t, ptok_t, oidx_t, ident_t, bmask_t,
          out_t, vocab, padw, bases, counts):
    dt = mybir.dt
    nwin = len(bases)
    ntokb = padw * nwin
    wranks = padw // P                # 7 table ranks kept per window
    sranks = WCALL // P               # 8 staging ranks gathered per window
    wcols = WCALL // 16
    oranks = NLOC // P                # 20 own-walk token ranks

    cst = ctx.enter_context(tc.tile_pool(name="cst", bufs=1))
    stg = ctx.enter_context(tc.tile_pool(name="stg", bufs=9))
    drm = ctx.enter_context(tc.tile_pool(name="drm", bufs=1, space="DRAM"))
    abp = ctx.enter_context(tc.tile_pool(name="apool", bufs=AB_BUFS))
    cp = (None if GRAM else
          ctx.enter_context(tc.tile_pool(name="cpool", bufs=6)))
    dp = ctx.enter_context(tc.tile_pool(name="dpool", bufs=D_BUFS))
    up = ctx.enter_context(tc.tile_pool(name="upool", bufs=UP_BUFS))
    scp = ctx.enter_context(tc.tile_pool(name="scr", bufs=3))
    ptp = ctx.enter_context(tc.tile_pool(name="ptp", bufs=4, space="PSUM"))
    pgr = ctx.enter_context(tc.tile_pool(name="pgr", bufs=2, space="PSUM"))

    dtab = drm.tile([ntokb + 3, 2 * EMB_DIM], dt.float8e4)
    widx = cst.tile([P, nwin * wcols], dt.int16)
    gidx = cst.tile([P, N_LISTS * IDX_COLS], dt.int16)
    strips = []
    for si_ in range(N_STREAMS):
        strip = cst.tile([P, NCOLS], dt.float32, tag=f"strip{si_}", name=f"strip{si_}")
        strips.append(strip)

    nc.sync.dma_start(widx[:], widx_t[:])
    nc.sync.dma_start(gidx[:], gidx_t[:])
    if GRAM:
        oidx = cst.tile([P, sum(OWN_CALLS) // 16], dt.int16)
        ident = cst.tile([P, P], dt.bfloat16)
        bmask = cst.tile([WALK_LEN, 12 * WALK_LEN], dt.float32)
        nc.sync.dma_start(oidx[:], oidx_t[:])
        nc.sync.dma_start(ident[:], ident_t[:])
        nc.sync.dma_start(bmask[:], bmask_t[:])

    qrr = [0]
    prev_g = [None]

    def nextq():
        q = qrr[0] % QUEUES
        qrr[0] += 1
        return q

    def chain(inst):
        # Pin scheduler emission order of SWDGE ops to program order so
        # Tile's DMA sem lanes (rr mod 8) stay aligned with the strict
        # queue round-robin (mod 4): ucode requires each sem lane to be
        # incremented from a single queue.
        if prev_g[0] is not None:
            add_dep_helper(inst.ins, prev_g[0].ins, False,
                           "swdge order chain")
        prev_g[0] = inst

    # Token-table build. Window i: one negative-padded 1024-idx single-packet
    # gather from its HBM slice into a rotating staging tile (parallel across
    # queues), then a bulk DMA of the first 896 slots into the DRAM table.
    nc.sync.dma_start(
        dtab[ntokb:ntokb + 3, :].rearrange("(r p) e -> p r e", p=3), ptok_t[:])
    for i in range(nwin):
        lo = bases[i]
        hi = min(vocab, lo + WIN_ROWS)
        s = stg.tile([P, sranks, 2 * EMB_DIM], dt.float8e4, tag="stg")
        g = nc.gpsimd.dma_gather(
            s[:], tok_t[lo:hi, :],
            widx[:, i * wcols:(i + 1) * wcols],
            WCALL, max(1, counts[i]), 2 * EMB_DIM,
            single_packet=True,
            queue_num=nextq(),
        )
        chain(g)
        nc.sync.dma_start(
            dtab[padw * i:padw * (i + 1), :].rearrange("(r p) e -> p r e", p=P),
            s[:, :wranks, :])

    def gather(dst, cols, n):
        g = nc.gpsimd.dma_gather(
            dst[:], dtab[:, :], gidx[:, cols:cols + n // 16], n, n,
            2 * EMB_DIM,
            single_packet=True,
            queue_num=nextq(),
        )
        chain(g)

    gavs = []
    if GRAM and not NO_COMPUTE:
        # own-walk tokens -> transposed bf16 node/ctx panels [128d, 2560pos]
        own8 = cst.tile([P, oranks, 2 * EMB_DIM], dt.float8e4)
        got = 0
        for n in OWN_CALLS:
            g = nc.gpsimd.dma_gather(
                own8[:, got // P:(got + n) // P, :], dtab[:, :],
                oidx[:, got // 16:(got + n) // 16], n, n, 2 * EMB_DIM,
                single_packet=True, queue_num=nextq())
            chain(g)
            got += n
        own_bf = cst.tile([P, oranks, 2 * EMB_DIM], dt.bfloat16)
        nc.scalar.activation(own_bf[:], own8[:],
                             mybir.ActivationFunctionType.Copy)
        nodeT = cst.tile([P, NLOC], dt.bfloat16)
        ctxT = cst.tile([P, NLOC], dt.bfloat16)
        for r in range(oranks):
            for h, dest in ((0, nodeT), (1, ctxT)):
                tp = ptp.tile([P, P], dt.bfloat16, tag="tp")
                nc.tensor.transpose(
                    tp[:], own_bf[:, r, h * EMB_DIM:(h + 1) * EMB_DIM],
                    ident[:])
                nc.scalar.activation(dest[:, r * P:(r + 1) * P], tp[:],
                                     mybir.ActivationFunctionType.Copy)
        # per-walk 40x40 grams, 12 walks per PSUM tile
        for g0 in range(0, NB_CORE, 12):
            nw = min(12, NB_CORE - g0)
            gps = pgr.tile([WALK_LEN, 12 * WALK_LEN], dt.float32, tag="gps")
            for s in range(nw):
                w = g0 + s
                nc.tensor.matmul(
                    gps[:, s * WALK_LEN:(s + 1) * WALK_LEN],
                    ctxT[:, w * WALK_LEN:(w + 1) * WALK_LEN],
                    nodeT[:, w * WALK_LEN:(w + 1) * WALK_LEN],
                    start=True, stop=True)
            ncol = nw * WALK_LEN
            gsb = scp.tile([WALK_LEN, 12 * WALK_LEN], dt.float32, tag="gsb")
            nc.vector.tensor_add(gsb[:, :ncol], gps[:, :ncol], bmask[:, :ncol])
            ge = scp.tile([WALK_LEN, 12 * WALK_LEN], dt.float32, tag="ge")
            nc.scalar.activation(ge[:, :ncol], gsb[:, :ncol],
                                 mybir.ActivationFunctionType.Exp,
                                 scale=-SCALE2)
            gsp = scp.tile([WALK_LEN, 12 * WALK_LEN], dt.float32, tag="gsp")
            gav = cst.tile([WALK_LEN, 1], dt.float32, tag=f"gav{g0}",
                           name=f"gav{g0}")
            nc.scalar.activation(gsp[:, :ncol], ge[:, :ncol],
                                 mybir.ActivationFunctionType.Ln,
                                 bias=1.0, accum_out=gav[:])
            gavs.append(gav)

    col0 = 0
    scol = 0
    for gi, n in enumerate(CALLS if not NO_MAIN else []):
        nr = n // P
        ab = abp.tile([P, nr, 2 * EMB_DIM], dt.float8e4, tag="ab")
        gather(ab, col0, n)
        if not GRAM:
            c8 = cp.tile([P, nr, 2 * EMB_DIM], dt.float8e4, tag="c8")
            gather(c8, IDX_COLS + col0, n)
        d8s = []
        for j in range(NEG_SIZE):
            d8 = dp.tile([P, nr, 2 * EMB_DIM], dt.float8e4, tag="d8")
            gather(d8, (2 + j) * IDX_COLS + col0, n)
            d8s.append(d8)
        if not NO_COMPUTE:
            # scalar-engine upcasts fp8 -> bf16
            a = up.tile([P, nr, EMB_DIM], dt.bfloat16, tag="a")
            nc.scalar.activation(a[:], ab[:, :, :EMB_DIM],
                                 mybir.ActivationFunctionType.Copy)
            if not GRAM:
                b = up.tile([P, nr, EMB_DIM], dt.bfloat16, tag="b")
                nc.scalar.activation(b[:], ab[:, :, EMB_DIM:],
                                     mybir.ActivationFunctionType.Copy)
                c = up.tile([P, nr, EMB_DIM], dt.bfloat16, tag="c")
                nc.scalar.activation(c[:], c8[:, :, :EMB_DIM],
                                     mybir.ActivationFunctionType.Copy)
                nc.vector.tensor_mul(c[:], c[:], b[:])
                nc.vector.tensor_reduce(strips[0][:, scol:scol + nr], c[:],
                                        axis=mybir.AxisListType.X,
                                        op=mybir.AluOpType.add)
            for j in range(NEG_SIZE):
                d = up.tile([P, nr, EMB_DIM], dt.bfloat16, tag="d")
                nc.scalar.activation(d[:], d8s[j][:, :, EMB_DIM:],
                                     mybir.ActivationFunctionType.Copy)
                nc.vector.tensor_mul(d[:], d[:], a[:])
                nc.vector.tensor_reduce(strips[1 + j][:, scol:scol + nr], d[:],
                                        axis=mybir.AxisListType.X,
                                        op=mybir.AluOpType.add)
        col0 += n // 16
        scol += nr

    accvs = []
    s2_first = 1 if GRAM else 0
    for s2 in range(s2_first,
                    N_STREAMS if not NO_COMPUTE and not NO_MAIN else s2_first):
        scale = -SCALE2 if s2 == 0 else SCALE2   # pos stream: softplus(-score)
        e = scp.tile([P, NCOLS], dt.float32, tag="e")
        sp = scp.tile([P, NCOLS], dt.float32, tag="sp")
        av = cst.tile([P, 1], dt.float32, tag=f"av{s2}")
        nc.scalar.activation(e[:], strips[s2][:],
                             mybir.ActivationFunctionType.Exp, scale=scale)
        nc.scalar.activation(sp[:], e[:],
                             mybir.ActivationFunctionType.Ln,
                             bias=1.0, accum_out=av[:])
        accvs.append(av)
    osb = cst.tile([P, 1], dt.float32, tag="osb")
    nc.vector.memset(osb[:], 0.0)
    for av in accvs:
        nc.vector.tensor_add(osb[:], osb[:], av[:])
    for gav in gavs:
        nc.vector.tensor_add(osb[:WALK_LEN, :], osb[:WALK_LEN, :], gav[:])
    nc.sync.dma_start(out_t[:], osb[:])


def _build_program(loop_k, vocab, padw, bases, counts):
    nc = bacc.Bacc("TRN2", target_bir_lowering=False, debug=False,
                   num_swdge_queues=QUEUES)
    nwin = len(bases)
    tok_t = nc.dram_tensor("tok", [vocab, 2 * EMB_DIM], mybir.dt.float8e4,
                           kind="ExternalInput")
    widx_t = nc.dram_tensor("widx", [P, nwin * WCALL // 16], mybir.dt.int16,
                            kind="ExternalInput")
    gidx_t = nc.dram_tensor("gidx", [P, N_LISTS * IDX_COLS], mybir.dt.int16,
                            kind="ExternalInput")
    ptok_t = nc.dram_tensor("ptok", [3, 2 * EMB_DIM], mybir.dt.float8e4,
                            kind="ExternalInput")
    oidx_t = nc.dram_tensor("oidx", [P, sum(OWN_CALLS) // 16], mybir.dt.int16,
                            kind="ExternalInput")
    ident_t = nc.dram_tensor("ident", [P, P], mybir.dt.bfloat16,
                             kind="ExternalInput")
    bmask_t = nc.dram_tensor("bmask", [WALK_LEN, 12 * WALK_LEN],
                             mybir.dt.float32, kind="ExternalInput")
    out_t = nc.dram_tensor("out", [P, 1], mybir.dt.float32, kind="ExternalOutput")
    with tile.TileContext(nc) as tc:
        nc.gpsimd.load_library(library_config.mlp)
        if loop_k is None:
            _body(tc, nc, tok_t, widx_t, gidx_t, ptok_t, oidx_t, ident_t,
                  bmask_t, out_t, vocab, padw, bases, counts)
        else:
            tc.For_i_unrolled(0, loop_k, 1,
                              lambda iv: _body(tc, nc, tok_t, widx_t, gidx_t,
                                               ptok_t, oidx_t, ident_t,
                                               bmask_t, out_t, vocab, padw,
                                               bases, counts),
                              max_unroll=1)
    nc.compile()
    return nc


_CACHE = {}


def _get_program(loop_k, vocab, padw, bases, counts):
    key = (loop_k, vocab, padw, bases, counts, NO_COMPUTE, NO_MAIN, QUEUES, GRAM, AB_BUFS, D_BUFS, UP_BUFS)
    if key not in _CACHE:
        _CACHE[key] = _build_program(loop_k, vocab, padw, bases, counts)
    return _CACHE[key]


def run_cores(inputs, loop_k=None):
    """Run the SPMD kernel; returns list of per-core [128,1] partial sums."""
    in_maps, padw, bases, counts = _host_prepare(**inputs)
    vocab = int(np.asarray(inputs["node_embed"]).shape[0])
    nc = _get_program(loop_k, vocab, padw, bases, counts)
    res = run_bass_kernel_spmd(nc, in_maps, core_ids=list(range(N_CORES)))
    return [res.results[i]["out"] for i in range(N_CORES)]


def kernel(batch_walk, neg_idx_dst, node_embed, context_embed):
    outs = run_cores(dict(batch_walk=batch_walk, neg_idx_dst=neg_idx_dst,
                          node_embed=node_embed, context_embed=context_embed))
    total = float(sum(float(o.sum()) for o in outs))
    return np.float32(total / (BATCH * NUM_PAIRS))


# revision 19
# speedup vs baseline: 1.3831x; 1.3831x over previous
"""DeepWalk loss kernel for 8 Trainium2 NeuronCores.

Strategy: data-parallel over the 512 walks (64 walks per core). Each core
compacts the referenced embedding rows into a DRAM token table in FP8
(node||ctx, 256B per token, values pre-scaled by S=128) via windowed
dma_gather (windows of 32768 rows so indices fit int16; negative-padded
single-packet calls so pad slots emit no descriptors). Pair operands are
fetched with 1024-index single-packet HBM dma_gather calls in row layout
(pairs on partitions) at 128B/256B per descriptor — half the bytes of the
bf16 variant, and the gathers are bandwidth-bound. Gathered fp8 tiles are
upcast to bf16 on the scalar engine, multiplied on DVE (2x mode) and
reduced along the embedding axis into per-stream score strips; softplus is
evaluated with the Exp and Ln LUTs on the scalar engine with the 1/S^2
score rescale folded into the Exp scale and a fused row-sum accumulator.
Host sums the 8x[128] partial sums and divides by the pair count.
"""

import os
import sys

import numpy as np
import ml_dtypes

sys.path.insert(0, "/opt/trn_rl_repo")

import concourse.bacc as bacc
import concourse.bass as bass
import concourse.mybir as mybir
import concourse.tile as tile
from concourse import library_config
from concourse.bass_utils import run_bass_kernel_spmd
from concourse._compat import with_exitstack
from concourse.tile import add_dep_helper

BF16 = ml_dtypes.bfloat16
E8 = ml_dtypes.float8_e4m3

# Problem constants (hardcoded per the harness contract).
EMB_DIM = 128
WALK_LEN = 40
WINDOW = 5
NEG_SIZE = 5
NUM_NODES = 1_000_000
BATCH = 512
N_CORES = 8

NB_CORE = BATCH // N_CORES            # 64 walks per core
NLOC = NB_CORE * WALK_LEN             # 2560 local walk positions
GTOK = BATCH * WALK_LEN               # 20480 global walk positions
P = 128
WIN_ROWS = 32768                      # int16-addressable window of the table
WCALL = 1024                          # idx per gather call (single-packet max)
FSCALE = 128.0                        # fp8 pre-scale of embedding values
PADV = 448.0                          # fp8 max-magnitude pad value
SCALE2 = 1.0 / (FSCALE * FSCALE)      # score rescale folded into Act

def _pair_indices():
    src, dst = [], []
    for i in range(WALK_LEN):
        for j in range(max(0, i - WINDOW), i):
            src.append(j); dst.append(i)
        for j in range(i + 1, min(WALK_LEN, i + 1 + WINDOW)):
            src.append(j); dst.append(i)
    return np.asarray(src, dtype=np.int64), np.asarray(dst, dtype=np.int64)

_SRC, _DST = _pair_indices()
NUM_PAIRS = _SRC.shape[0]             # 370
POS_CORE = NB_CORE * NUM_PAIRS        # 23680 positive pairs per core
NPAD = 24064                          # pairs per stream, padded to 128 (188 cols)
NCOLS = NPAD // P                     # 188 score columns per stream
CALLS = [WCALL] * (NPAD // WCALL) + ([NPAD % WCALL] if NPAD % WCALL else [])
N_STREAMS = 6                         # pos, neg j=0..4
IDX_COLS = NPAD // 16                 # 1504 idx columns per list
N_LISTS = 7                           # AB, C, D0..D4

# --- experiment knobs (timing/ablation; defaults = production) ---
NO_COMPUTE = False      # skip mult/reduce/softplus (gathers only)
NO_MAIN = False         # skip the main pair gathers (build only)
QUEUES = 4              # SWDGE queues to round-robin gathers over
FAKE_WINDOWS = 0        # timing mode: N fake windows over a small vocab
GRAM = True             # pos scores via per-walk TensorE Gram (no C gathers)
BIGB = 900.0 / SCALE2   # mask bias: softplus(-(s+BIGB)*SCALE2) ~= 0
OWN_CALLS = [1024, 1024, 512]  # own-walk token gather calls (2560 tokens)
AB_BUFS = 8             # AB-tile pool depth (gather pipeline)
D_BUFS = 18             # D-tile pool depth
UP_BUFS = 10            # upcast-tile pool depth


def _wrap16(a):
    """int16 list [N] -> [128, N/16] dma_gather idx layout (16-wrap, 8x replicated)."""
    a = a.astype(np.int16)
    t = a.reshape(-1, 16).T          # [16, N/16]
    return np.tile(t, (8, 1)).copy() # [128, N/16]


def _plan(fw, vocab):
    """Window build plan: group the 20480 walk rows by table window.

    Each window is gathered with one WCALL(=1024)-index single-packet call
    straight into DRAM-table staging; trailing indices are -1 (descriptor
    skipped). Token ids are padw*i + slot.

    Returns (padw, bases, widx_lists, counts, pos).
    """
    if FAKE_WINDOWS:
        nwin = FAKE_WINDOWS
        win = np.argsort(np.argsort(fw, kind="stable"), kind="stable") % nwin
        bases = [0] * nwin
    else:
        nwin = (vocab + WIN_ROWS - 1) // WIN_ROWS
        win = fw // WIN_ROWS
        bases = [WIN_ROWS * i for i in range(nwin)]
    counts = np.bincount(win, minlength=nwin)
    padw = 896
    assert counts.max() <= min(padw, WCALL), f"window overflow: {counts.max()}"
    pos = np.empty(GTOK, dtype=np.int32)
    widx_lists = []
    for i in range(nwin):
        ws = np.nonzero(win == i)[0]
        pos[ws] = padw * i + np.arange(len(ws), dtype=np.int32)
        lst = np.full(WCALL, -1, dtype=np.int32)
        lst[:len(ws)] = fw[ws] - bases[i]
        widx_lists.append(lst)
    return padw, bases, widx_lists, tuple(int(c) for c in counts), pos


def _host_prepare(batch_walk, neg_idx_dst, node_embed, context_embed):
    """Sharding/index prep. Index arithmetic + dtype casts only."""
    fw = np.asarray(batch_walk).reshape(-1).astype(np.int32)       # [20480]
    neg = np.asarray(neg_idx_dst).astype(np.int32)                 # [947200]
    vocab = int(np.asarray(node_embed).shape[0])

    tok = np.empty((vocab, 2 * EMB_DIM), dtype=E8)
    tok[:, :EMB_DIM] = (np.asarray(node_embed) * FSCALE).astype(E8)
    tok[:, EMB_DIM:] = (np.asarray(context_embed) * FSCALE).astype(E8)

    padw, bases, widx_lists, counts, pos = _plan(fw, vocab)
    nwin = len(bases)
    ntokb = padw * nwin
    pad_x, pad_y, pad_z = ntokb, ntokb + 1, ntokb + 2
    assert ntokb + 3 <= 32768, "token ids must fit int16"

    # pads: AB->X, C->Y, D->Z.  pos pad score = node_Y*ctx_X = +PADV^2;
    # neg pad score = node_X*ctx_Z = -PADV^2. softplus(-PADV^2/S^2) ~= 0.
    ptok = np.zeros((3, 2 * EMB_DIM), dtype=E8)
    ptok[0, 0] = PADV; ptok[0, EMB_DIM] = PADV   # X: node=448, ctx=448
    ptok[1, 0] = PADV                            # Y: node=448
    ptok[2, EMB_DIM] = -PADV                     # Z: ctx=-448

    widx = np.concatenate([_wrap16(a) for a in widx_lists], axis=1)

    bl = np.repeat(np.arange(NB_CORE, dtype=np.int32), NUM_PAIRS)
    qq = np.tile(np.arange(NUM_PAIRS, dtype=np.int32), NB_CORE)
    npad = NPAD - POS_CORE

    # constants for the Gram pos path
    ident = np.eye(P, dtype=BF16)
    ii, jj = np.meshgrid(np.arange(WALK_LEN), np.arange(WALK_LEN), indexing="ij")
    m40 = (np.abs(ii - jj) <= WINDOW) & (ii != jj)       # valid (dst,src) pairs
    bmask = np.where(np.tile(~m40, (1, 12)), BIGB, 0.0).astype(np.float32)

    in_maps = []
    for k in range(N_CORES):
        wloc = k * NLOC  # this core's batches start at walk position k*2560
        ab_t = pos[wloc + bl * WALK_LEN + _DST[qq].astype(np.int32)]
        c_t = pos[wloc + bl * WALK_LEN + _SRC[qq].astype(np.int32)]
        ab = np.concatenate([ab_t, np.full(npad, pad_x, np.int32)])
        cc = np.concatenate([c_t, np.full(npad, pad_y, np.int32)])
        negk = neg[k * POS_CORE * NEG_SIZE:(k + 1) * POS_CORE * NEG_SIZE]
        negk = negk.reshape(POS_CORE, NEG_SIZE)
        lists = [ab, cc]
        for j in range(NEG_SIZE):
            dj = np.concatenate([pos[negk[:, j]], np.full(npad, pad_z, np.int32)])
            lists.append(dj)
        gidx = np.concatenate([_wrap16(a) for a in lists], axis=1)  # [128, 7*1504]
        own = np.full(sum(OWN_CALLS), -1, np.int32)
        own[:NLOC] = pos[wloc:wloc + NLOC]
        oidx = _wrap16(own)
        in_maps.append({"tok": tok, "widx": widx, "gidx": gidx, "ptok": ptok,
                        "oidx": oidx, "ident": ident, "bmask": bmask})
    return in_maps, padw, tuple(bases), counts


@with_exitstack
def _body(ctx, tc, nc, tok_t, widx_t, gidx_t, ptok_t, oidx_t, ident_t, bmask_t,
          out_t, vocab, padw, bases, counts):
    dt = mybir.dt
    nwin = len(bases)
    ntokb = padw * nwin
    wranks = padw // P                # 7 table ranks kept per window
    sranks = WCALL // P               # 8 staging ranks gathered per window
    wcols = WCALL // 16
    oranks = NLOC // P                # 20 own-walk token ranks

    cst = ctx.enter_context(tc.tile_pool(name="cst", bufs=1))
    stg = ctx.enter_context(tc.tile_pool(name="stg", bufs=8))
    drm = ctx.enter_context(tc.tile_pool(name="drm", bufs=1, space="DRAM"))
    abp = ctx.enter_context(tc.tile_pool(name="apool", bufs=AB_BUFS))
    cp = (None if GRAM else
          ctx.enter_context(tc.tile_pool(name="cpool", bufs=6)))
    dp = ctx.enter_context(tc.tile_pool(name="dpool", bufs=D_BUFS))
    up = ctx.enter_context(tc.tile_pool(name="upool", bufs=UP_BUFS))
    scp = ctx.enter_context(tc.tile_pool(name="scr", bufs=3))
    ptp = ctx.enter_context(tc.tile_pool(name="ptp", bufs=4, space="PSUM"))
    pgr = ctx.enter_context(tc.tile_pool(name="pgr", bufs=2, space="PSUM"))

    dtab = drm.tile([ntokb + 3, 2 * EMB_DIM], dt.float8e4)
    widx = cst.tile([P, nwin * wcols], dt.int16)
    gidx = cst.tile([P, N_LISTS * IDX_COLS], dt.int16)
    strips = []
    for si_ in range(N_STREAMS):
        strip = cst.tile([P, NCOLS], dt.float32, tag=f"strip{si_}", name=f"strip{si_}")
        strips.append(strip)

    nc.sync.dma_start(widx[:], widx_t[:])
    nc.sync.dma_start(gidx[:], gidx_t[:])
    if GRAM:
        oidx = cst.tile([P, sum(OWN_CALLS) // 16], dt.int16)
        ident = cst.tile([P, P], dt.bfloat16)
        bmask = cst.tile([WALK_LEN, 12 * WALK_LEN], dt.float32)
        nc.sync.dma_start(oidx[:], oidx_t[:])
        nc.sync.dma_start(ident[:], ident_t[:])
        nc.sync.dma_start(bmask[:], bmask_t[:])

    qrr = [0]
    prev_g = [None]

    def nextq():
        q = qrr[0] % QUEUES
        qrr[0] += 1
        return q

    def chain(inst):
        # Pin scheduler emission order of SWDGE ops to program order so
        # Tile's DMA sem lanes (rr mod 8) stay aligned with the strict
        # queue round-robin (mod 4): ucode requires each sem lane to be
        # incremented from a single queue.
        if prev_g[0] is not None:
            add_dep_helper(inst.ins, prev_g[0].ins, False,
                           "swdge order chain")
        prev_g[0] = inst

    # Token-table build. Window i: one negative-padded 1024-idx single-packet
    # gather from its HBM slice into a rotating staging tile (parallel across
    # queues), then a bulk DMA of the first 896 slots into the DRAM table.
    nc.sync.dma_start(
        dtab[ntokb:ntokb + 3, :].rearrange("(r p) e -> p r e", p=3), ptok_t[:])
    for i in range(nwin):
        lo = bases[i]
        hi = min(vocab, lo + WIN_ROWS)
        s = stg.tile([P, sranks, 2 * EMB_DIM], dt.float8e4, tag="stg")
        g = nc.gpsimd.dma_gather(
            s[:], tok_t[lo:hi, :],
            widx[:, i * wcols:(i + 1) * wcols],
            WCALL, max(1, counts[i]), 2 * EMB_DIM,
            single_packet=True,
            queue_num=nextq(),
        )
        chain(g)
        nc.sync.dma_start(
            dtab[padw * i:padw * (i + 1), :].rearrange("(r p) e -> p r e", p=P),
            s[:, :wranks, :])

    def gather(dst, cols, n):
        g = nc.gpsimd.dma_gather(
            dst[:], dtab[:, :], gidx[:, cols:cols + n // 16], n, n,
            2 * EMB_DIM,
            single_packet=True,
            queue_num=nextq(),
        )
        chain(g)

    gavs = []
    if GRAM and not NO_COMPUTE:
        # own-walk tokens -> transposed bf16 node/ctx panels [128d, 2560pos]
        own8 = cst.tile([P, oranks, 2 * EMB_DIM], dt.float8e4)
        got = 0
        for n in OWN_CALLS:
            g = nc.gpsimd.dma_gather(
                own8[:, got // P:(got + n) // P, :], dtab[:, :],
                oidx[:, got // 16:(got + n) // 16], n, n, 2 * EMB_DIM,
                single_packet=True, queue_num=nextq())
            chain(g)
            got += n
        own_bf = cst.tile([P, oranks, 2 * EMB_DIM], dt.bfloat16)
        nc.scalar.activation(own_bf[:], own8[:],
                             mybir.ActivationFunctionType.Copy)
        nodeT = cst.tile([P, NLOC], dt.bfloat16)
        ctxT = cst.tile([P, NLOC], dt.bfloat16)
        for r in range(oranks):
            for h, dest in ((0, nodeT), (1, ctxT)):
                tp = ptp.tile([P, P], dt.bfloat16, tag="tp")
                nc.tensor.transpose(
                    tp[:], own_bf[:, r, h * EMB_DIM:(h + 1) * EMB_DIM],
                    ident[:])
                nc.scalar.activation(dest[:, r * P:(r + 1) * P], tp[:],
                                     mybir.ActivationFunctionType.Copy)
        # per-walk 40x40 grams, 12 walks per PSUM tile
        for g0 in range(0, NB_CORE, 12):
            nw = min(12, NB_CORE - g0)
            gps = pgr.tile([WALK_LEN, 12 * WALK_LEN], dt.float32, tag="gps")
            for s in range(nw):
                w = g0 + s
                nc.tensor.matmul(
                    gps[:, s * WALK_LEN:(s + 1) * WALK_LEN],
                    ctxT[:, w * WALK_LEN:(w + 1) * WALK_LEN],
                    nodeT[:, w * WALK_LEN:(w + 1) * WALK_LEN],
                    start=True, stop=True)
            ncol = nw * WALK_LEN
            gsb = scp.tile([WALK_LEN, 12 * WALK_LEN], dt.float32, tag="gsb")
            nc.vector.tensor_add(gsb[:, :ncol], gps[:, :ncol], bmask[:, :ncol])
            ge = scp.tile([WALK_LEN, 12 * WALK_LEN], dt.float32, tag="ge")
            nc.scalar.activation(ge[:, :ncol], gsb[:, :ncol],
                                 mybir.ActivationFunctionType.Exp,
                                 scale=-SCALE2)
            gsp = scp.tile([WALK_LEN, 12 * WALK_LEN], dt.float32, tag="gsp")
            gav = cst.tile([WALK_LEN, 1], dt.float32, tag=f"gav{g0}",
                           name=f"gav{g0}")
            nc.scalar.activation(gsp[:, :ncol], ge[:, :ncol],
                                 mybir.ActivationFunctionType.Ln,
                                 bias=1.0, accum_out=gav[:])
            gavs.append(gav)

    col0 = 0
    scol = 0
    for gi, n in enumerate(CALLS if not NO_MAIN else []):
        nr = n // P
        ab = abp.tile([P, nr, 2 * EMB_DIM], dt.float8e4, tag="ab")
        gather(ab, col0, n)
        if not GRAM:
            c8 = cp.tile([P, nr, 2 * EMB_DIM], dt.float8e4, tag="c8")
            gather(c8, IDX_COLS + col0, n)
        d8s = []
        for j in range(NEG_SIZE):
            d8 = dp.tile([P, nr, 2 * EMB_DIM], dt.float8e4, tag="d8")
            gather(d8, (2 + j) * IDX_COLS + col0, n)
            d8s.append(d8)
        if not NO_COMPUTE:
            # scalar-engine upcasts fp8 -> bf16
            a = up.tile([P, nr, EMB_DIM], dt.bfloat16, tag="a")
            nc.scalar.activation(a[:], ab[:, :, :EMB_DIM],
                                 mybir.ActivationFunctionType.Copy)
            if not GRAM:
                b = up.tile([P, nr, EMB_DIM], dt.bfloat16, tag="b")
                nc.scalar.activation(b[:], ab[:, :, EMB_DIM:],
                                     mybir.ActivationFunctionType.Copy)
                c = up.tile([P, nr, EMB_DIM], dt.bfloat16, tag="c")
                nc.scalar.activation(c[:], c8[:, :, :EMB_DIM],
                                     mybir.ActivationFunctionType.Copy)
                nc.vector.tensor_mul(c[:], c[:], b[:])
                nc.vector.tensor_reduce(strips[0][:, scol:scol + nr], c[:],
                                        axis=mybir.AxisListType.X,
                                        op=mybir.AluOpType.add)
            for j in range(NEG_SIZE):
                d = up.tile([P, nr, EMB_DIM], dt.bfloat16, tag="d")
                nc.scalar.activation(d[:], d8s[j][:, :, EMB_DIM:],
                                     mybir.ActivationFunctionType.Copy)
                nc.vector.tensor_mul(d[:], d[:], a[:])
                nc.vector.tensor_reduce(strips[1 + j][:, scol:scol + nr], d[:],
                                        axis=mybir.AxisListType.X,
                                        op=mybir.AluOpType.add)
        col0 += n // 16
        scol += nr

    accvs = []
    s2_first = 1 if GRAM else 0
    for s2 in range(s2_first,
                    N_STREAMS if not NO_COMPUTE and not NO_MAIN else s2_first):
        scale = -SCALE2 if s2 == 0 else SCALE2   # pos stream: softplus(-score)
        e = scp.tile([P, NCOLS], dt.float32, tag="e")
        sp = scp.tile([P, NCOLS], dt.float32, tag="sp")
        av = cst.tile([P, 1], dt.float32, tag=f"av{s2}")
        nc.scalar.activation(e[:], strips[s2][:],
                             mybir.ActivationFunctionType.Exp, scale=scale)
        nc.scalar.activation(sp[:], e[:],
                             mybir.ActivationFunctionType.Ln,
                             bias=1.0, accum_out=av[:])
        accvs.append(av)
    osb = cst.tile([P, 1], dt.float32, tag="osb")
    nc.vector.memset(osb[:], 0.0)
    for av in accvs:
        nc.vector.tensor_add(osb[:], osb[:], av[:])
    for gav in gavs:
        nc.vector.tensor_add(osb[:WALK_LEN, :], osb[:WALK_LEN, :], gav[:])
    nc.sync.dma_start(out_t[:], osb[:])


def _build_program(loop_k, vocab, padw, bases, counts):
    nc = bacc.Bacc("TRN2", target_bir_lowering=False, debug=False,
                   num_swdge_queues=QUEUES)
    nwin = len(bases)
    tok_t = nc.dram_tensor("tok", [vocab, 2 * EMB_DIM], mybir.dt.float8e4,
                           kind="ExternalInput")
    widx_t = nc.dram_tensor("widx", [P, nwin * WCALL // 16], mybir.dt.int16,
                            kind="ExternalInput")
    gidx_t = nc.dram_tensor("gidx", [P, N_LISTS * IDX_COLS], mybir.dt.int16,
                            kind="ExternalInput")
    ptok_t = nc.dram_tensor("ptok", [3, 2 * EMB_DIM], mybir.dt.float8e4,
                            kind="ExternalInput")
    oidx_t = nc.dram_tensor("oidx", [P, sum(OWN_CALLS) // 16], mybir.dt.int16,
                            kind="ExternalInput")
    ident_t = nc.dram_tensor("ident", [P, P], mybir.dt.bfloat16,
                             kind="ExternalInput")
    bmask_t = nc.dram_tensor("bmask", [WALK_LEN, 12 * WALK_LEN],
                             mybir.dt.float32, kind="ExternalInput")
    out_t = nc.dram_tensor("out", [P, 1], mybir.dt.float32, kind="ExternalOutput")
    with tile.TileContext(nc) as tc:
        nc.gpsimd.load_library(library_config.mlp)
        if loop_k is None:
            _body(tc, nc, tok_t, widx_t, gidx_t, ptok_t, oidx_t, ident_t,
                  bmask_t, out_t, vocab, padw, bases, counts)
        else:
            tc.For_i_unrolled(0, loop_k, 1,
                              lambda iv: _body(tc, nc, tok_t, widx_t, gidx_t,
                                               ptok_t, oidx_t, ident_t,
                                               bmask_t, out_t, vocab, padw,
                                               bases, counts),
                              max_unroll=1)
    nc.compile()
    return nc


_CACHE = {}


def _get_program(loop_k, vocab, padw, bases, counts):
    key = (loop_k, vocab, padw, bases, counts, NO_COMPUTE, NO_MAIN, QUEUES, GRAM, AB_BUFS, D_BUFS, UP_BUFS)
    if key not in _CACHE:
        _CACHE[key] = _build_program(loop_k, vocab, padw, bases, counts)
    return _CACHE[key]


def run_cores(inputs, loop_k=None):
    """Run the SPMD kernel; returns list of per-core [128,1] partial sums."""
    in_maps, padw, bases, counts = _host_prepare(**inputs)
    vocab = int(np.asarray(inputs["node_embed"]).shape[0])
    nc = _get_program(loop_k, vocab, padw, bases, counts)
    res = run_bass_kernel_spmd(nc, in_maps, core_ids=list(range(N_CORES)))
    return [res.results[i]["out"] for i in range(N_CORES)]


def kernel(batch_walk, neg_idx_dst, node_embed, context_embed):
    outs = run_cores(dict(batch_walk=batch_walk, neg_idx_dst=neg_idx_dst,
                          node_embed=node_embed, context_embed=context_embed))
    total = float(sum(float(o.sum()) for o in outs))
    return np.float32(total / (BATCH * NUM_PAIRS))
